# revision 39
# baseline (speedup 1.0000x reference)
"""Causal flash attention (B=2, H=16, S=2048, D=64, fp32) on 8 TRN2 NeuronCores.

Strategy: shard batch*heads (32) across 8 cores -> 4 heads/core. Per head,
compute transposed scores S^T[k, q] = K Q^T via PE (fp16 inputs, fp32 PSUM
accumulate), exponentiate, then PV via PE with a ones column appended to V
so the softmax denominator falls out of the same matmul. The output leaves
the device transposed ([d+1, q] per head, fp32); the host divides by the
denominator row and transposes back.

Engine-level structure (all HW-traced):
 - The exp pass is an engine bottleneck: every score element must cross
   PSUM->SBUF through ScalarE or VectorE at 1 elem/lane/cycle. It is
   SPLIT between ScalarE (exact exp, scale folded into the activation)
   and VectorE (Schraudolph exp2 bit trick: i16 = rne(s*A + B) with
   A = 1024*SCALE*log2(e); bitcasting i16 to fp16 gives 2^t*(1+eps),
   |eps| <= 3%, geometric-mean centered -- softmax cancels the row-mean
   error component; measured output rel err ~7e-3 vs the 2e-2 budget).
   A greedy busy-time estimator routes each score group, and each
   PSUM->SBUF output copy, to whichever engine is less loaded.
 - Engine queues are strictly in-order, so stalls propagate: PV matmuls
   trail the exp stage by PV_LAG groups so the PE never reaches a PV
   whose p isn't ready; output copies are deferred one group so their
   PSUM waits are satisfied before entering the ACT/DVE streams; all
   input k/q DMA rides the sync queue and v the gpsimd queue (DMA
   instructions on a compute engine's queue would block its compute).
 - Per q-block (ascending), k-tiles go [clean pair first] -> [diagonal
   pair] -> [remaining clean pairs]: the diagonal exp->mask->PV chain
   hides under clean-group work and each q-block ends on a short clean
   tail. Causal masks are 0/1 multiplies on VectorE, batched across the
   two packed heads via strided APs; the second diagonal tile (fully
   masked left half) is restricted to its valid right 128 columns in
   QK, exp, and PV.
 - Two heads pack the 128 SBUF partitions (d=64 each) so QK matmuls for
   a head pair run concurrently on disjoint PE row groups; the pair's PV
   accumulators share one PSUM bank ([65, 512]; the second chain's first
   matmul uses start=False to avoid the bank-wide has_written clear).
   V is host-prepacked partition-major so each SBUF partition reads one
   contiguous HBM run.
"""

import numpy as np

B, H, S, D = 2, 16, 2048, 64
BH = B * H
NCORES = 8
HPC = BH // NCORES  # heads per core
SCALE = 0.125
W = 256             # q-block width (matmul moving dim)
TK = 128            # k-tile height
NKT = S // TK       # 16 k-tiles
NQB = S // W        # 8 q-blocks
G = 2               # k-tiles per exp group; [128, 2*G*W] fp32 = 2 PSUM banks
PV_LAG = 12         # groups the PV stage trails the exp stage by

LOG2E = 1.4426950408889634
A_C = 1024.0 * SCALE * LOG2E      # Schraudolph scale
B_C = 15360.0 - 58.7              # (15<<10) + geometric-mean centering
DVE_FRAC = 0.52                   # share of off-diag exp groups on VectorE

_CACHE = {}


def _build_nc():
    import concourse.bass as bass  # noqa: F401
    import concourse.mybir as mybir
    import concourse.tile as tile
    from concourse import bacc

    f32 = mybir.dt.float32
    f16 = mybir.dt.float16
    i16 = mybir.dt.int16
    EXP = mybir.ActivationFunctionType.Exp
    MULT = mybir.AluOpType.mult
    ADD = mybir.AluOpType.add

    nc = bacc.Bacc("TRN2", target_bir_lowering=False, debug=False, num_devices=NCORES)

    qt_d = nc.dram_tensor("qt", [HPC, D, S], f16, kind="ExternalInput").ap()
    kt_d = nc.dram_tensor("kt", [HPC, D, S], f16, kind="ExternalInput").ap()
    # v arrives host-packed as [HPC, 128, NKT*(D+1)]: partition-major so each
    # SBUF partition reads one contiguous run; ones column pre-appended.
    v_d = nc.dram_tensor("v", [HPC, 128, NKT * (D + 1)], f16, kind="ExternalInput").ap()
    o_d = nc.dram_tensor("outT", [HPC, D + 1, S], f32, kind="ExternalOutput").ap()

    with tile.TileContext(nc) as tc:
        const_pool = tc.alloc_tile_pool(name="const", bufs=1)
        kq_pool = tc.alloc_tile_pool(name="kq", bufs=1)
        vx_pool = tc.alloc_tile_pool(name="vx", bufs=1)
        p_pool = tc.alloc_tile_pool(name="p", bufs=18)
        o_pool = tc.alloc_tile_pool(name="o", bufs=4)
        ps_pool = tc.alloc_tile_pool(name="ps", bufs=3, space="PSUM")
        pv_pool = tc.alloc_tile_pool(name="pv", bufs=2, space="PSUM")

        # Causal masks, duplicated for the two packed heads so mask APs need
        # no broadcast dims: maskA2 = [maskA | maskA] with
        # maskA[x, y] = 1 if y >= x else 0 (A-diagonal tiles); B-diagonal
        # right halves use maskAL2 = [maskA[:, :128] | maskA[:, :128]].
        maskA2 = const_pool.tile([128, 2 * W], f16, tag="maskA2")
        maskAL2 = const_pool.tile([128, 2 * 128], f16, tag="maskAL2")
        for m, w in ((maskA2, W), (maskAL2, 128)):
            nc.gpsimd.memset(m[:], 1.0)
            for half in range(2):
                nc.gpsimd.affine_select(
                    out=m[:, half * w:(half + 1) * w],
                    in_=m[:, half * w:(half + 1) * w],
                    compare_op=mybir.AluOpType.is_ge,
                    fill=0.0, base=0,
                    pattern=[[1, w]], channel_multiplier=-1,
                )

        # Warm the ACT exp table during the input-DMA wait.
        warm = const_pool.tile([128, 1], f32, tag="warm")
        nc.gpsimd.memset(warm[:], 0.0)
        nc.scalar.activation(warm[:], warm[:], EXP, scale=SCALE)

        # Warm the PE HAM clock-gate during the input-DMA wait: ~4us of
        # dummy matmuls on the mask constants flip the PE to 8/8 (2.4GHz)
        # before the first real QK arrives (cold MMs run ~2x slower).
        pe_warm = pv_pool.tile([D + 1, 2 * W], f32, tag="pv", name="warmup")
        for _ in range(40):
            nc.tensor.matmul(
                pe_warm[0:64, 0:64], maskA2[0:64, 0:64],
                maskA2[0:64, 64:128], start=True, stop=True,
                skip_group_check=True,
            )

        # Input loads. kt/qt are packed 2 heads per 128 partitions and
        # chunked ascending (consumption order). k/q ride the sync DMA
        # queue, v the gpsimd queue. DMA instructions execute in-order on
        # their queue, so none may sit on scalar/vector/tensor.
        ktc = {}
        qtc = {}
        vxc = {}
        for pr in range(2):
            hA, hB = 2 * pr, 2 * pr + 1
            kchunk = kq_pool.tile([128, S], f16, tag=f"ktc{pr}", name=f"ktc{pr}")
            qchunk = kq_pool.tile([128, S], f16, tag=f"qtc{pr}", name=f"qtc{pr}")
            ktc[pr] = kchunk
            qtc[pr] = qchunk
            for h in (hA, hB):
                vchunk = vx_pool.tile([128, NKT * (D + 1)], f16, tag=f"vx{h}",
                                      name=f"vx{h}")
                nc.gpsimd.dma_start(vchunk[:], v_d[h])
                vxc[h] = vchunk
        kslices = (slice(0, 512), slice(512, 1024), slice(1024, 1536),
                   slice(1536, S))
        qslices = (slice(0, 512), slice(512, 1024), slice(1024, 1536),
                   slice(1536, S))
        for pr in range(2):
            hsl = slice(2 * pr, 2 * pr + 2)
            for ks, qs in zip(kslices, qslices):
                nc.sync.dma_start(
                    ktc[pr][:, ks],
                    kt_d[hsl, :, ks].rearrange("h d s -> (h d) s"),
                )
                nc.sync.dma_start(
                    qtc[pr][:, qs],
                    qt_d[hsl, :, qs].rearrange("h d s -> (h d) s"),
                )

        def ktile(pr, kt):
            return ktc[pr][:, kt * TK:(kt + 1) * TK]

        def vx(h, kt):
            return vxc[h][:, kt * (D + 1):(kt + 1) * (D + 1)]

        # greedy engine-load balancing (ns estimates from HW traces)
        est = {"act": 0.0, "dve": 0.0}
        ACT_G, DVE_G = 1113.0, 1224.0   # clean [128,1024] group exp
        ACT_DIAG = 1160.0               # two strided diag activations
        ACT_CP, DVE_CP = 720.0, 687.0   # [65,512] PSUM->SBUF copy

        def out_dma(dst, src):
            # outs ride the sync queue only: the gpsimd queue must stay
            # clear for the causal-mask multiplies (in-order queues).
            nc.sync.dma_start(dst, src)

        # Main pipeline, one head-pair at a time. Per q-block the k-tiles
        # are processed diagonal-pair first, then ascending: score groups
        # are [128, 2*G*W] (2 PSUM banks), triple-buffered; PV matmuls lag
        # one group behind the exp. Head A occupies group cols [0, gw*W),
        # head B [gw*W, 2*gw*W). pvAB holds both heads' accumulators in one
        # PSUM bank ([65, 2W]: A cols [0,W), B cols [W,2W)).
        pending_out = []  # deferred (qb, hA, hB, pvAB) output copies

        def drain_outs():
            # Emitted one group AFTER the q-block's last PV matmuls were
            # emitted, so the copy's semaphore wait is (nearly) satisfied by
            # the time its engine reaches it — it must not stall the
            # in-order ACT/DVE streams that also carry the exp work.
            while pending_out:
                qb, hA_, hB_, pvAB = pending_out.pop(0)
                oAB = o_pool.tile([D + 1, 2 * W], f32, tag="o")
                if est["dve"] + DVE_CP <= est["act"] + ACT_CP:
                    est["dve"] += DVE_CP
                    nc.vector.tensor_copy(oAB[:], pvAB[:])
                else:
                    est["act"] += ACT_CP
                    nc.scalar.copy(oAB[:], pvAB[:])
                out_dma(o_d[hA_, :, qb * W:(qb + 1) * W], oAB[:, 0:W])
                out_dma(o_d[hB_, :, qb * W:(qb + 1) * W], oAB[:, W:2 * W])

        for pr in range(2):
            hA, hB = 2 * pr, 2 * pr + 1
            pending = []  # up to PV_LAG of (qb, kts, p, pvAB, first, last)

            def flush_one():
                qb, kts, p, pvAB, first, last = pending.pop(0)
                nkt = 2 * qb + 2
                gw = len(kts)
                for j, kt in enumerate(kts):
                    bdiag = (kt == nkt - 1)
                    stop = last and j == gw - 1
                    for hoff, h in ((0, hA), (1, hB)):
                        poff = hoff * gw * W
                        pv = pvAB[:, hoff * W:(hoff + 1) * W]
                        if bdiag:
                            nc.tensor.matmul(
                                pv[:, 128:W],
                                vx(h, kt),
                                p[:, poff + j * W + 128:poff + (j + 1) * W],
                                start=False,
                                stop=stop,
                                skip_group_check=True,
                            )
                        else:
                            # start=True only on the very first matmul into
                            # the shared bank (head A); head B's first chain
                            # entry uses start=False so it does not re-clear
                            # the bank-wide has_written bits.
                            nc.tensor.matmul(
                                pv[:],
                                vx(h, kt),
                                p[:, poff + j * W:poff + (j + 1) * W],
                                start=(first and j == 0 and hoff == 0),
                                stop=stop,
                                skip_group_check=True,
                            )
                if last:  # last group of the q-block: defer the write-out
                    pending_out.append((qb, hA, hB, pvAB))

            def flush_pending(force=False):
                # PV matmuls run PV_LAG groups behind the exp stage so the
                # in-order PE stream never reaches a PV whose p isn't ready.
                drain_outs()
                keep = 0 if force else PV_LAG - 1
                while len(pending) > keep:
                    flush_one()

            for qb in range(NQB):
                nkt = 2 * qb + 2
                pvAB = pv_pool.tile([D + 1, 2 * W], f32, tag="pv", name="pvAB")
                qA = qtc[pr][0:64, qb * W:(qb + 1) * W]
                qB = qtc[pr][64:128, qb * W:(qb + 1) * W]
                # one clean group first (fast PV-chain start), then the
                # diagonal pair (its exp->mask latency hides under the
                # following clean groups), then the remaining clean tiles
                if nkt >= 6:
                    kt_order = [0, 1, nkt - 2, nkt - 1] + list(range(2, nkt - 2))
                else:
                    kt_order = list(range(nkt))
                groups = [kt_order[i:i + G] for i in range(0, nkt, G)]
                for gi, kts in enumerate(groups):
                    gw = len(kts)
                    diag = (nkt - 1) in kts
                    sG = ps_pool.tile([128, 2 * G * W], f32, tag="sG")
                    for j, kt in enumerate(kts):
                        bdiag = diag and j == 1
                        for hoff, (kt_half, q_half) in enumerate(
                            ((slice(0, 64), qA), (slice(64, 128), qB))
                        ):
                            coff = hoff * gw * W
                            if bdiag:
                                nc.tensor.matmul(
                                    sG[:, coff + j * W + 128:coff + (j + 1) * W],
                                    ktile(pr, kt)[kt_half], q_half[:, 128:W],
                                    start=True, stop=True,
                                )
                            else:
                                nc.tensor.matmul(
                                    sG[:, coff + j * W:coff + (j + 1) * W],
                                    ktile(pr, kt)[kt_half], q_half,
                                    start=True, stop=True,
                                )
                    p = p_pool.tile([128, 2 * G * W], f16, tag="p")
                    hw = gw * W
                    if not diag:
                        # whole group on one engine, greedy-balanced
                        if est["dve"] + DVE_G <= est["act"] + ACT_G:
                            est["dve"] += DVE_G
                            nc.vector.tensor_scalar(
                                p[:, :2 * hw].bitcast(i16), sG[:, :2 * hw],
                                A_C, B_C, MULT, ADD,
                            )
                        else:
                            est["act"] += ACT_G
                            nc.scalar.activation(
                                p[:, :2 * hw], sG[:, :2 * hw], EXP,
                                scale=SCALE,
                            )
                    else:
                        est["act"] += ACT_DIAG
                        s3 = sG[:].rearrange("c (h x) -> c h x", x=hw)
                        p3 = p[:].rearrange("c (h x) -> c h x", x=hw)
                        mA = maskA2[:].rearrange("c (h x) -> c h x", x=W)
                        mL = maskAL2[:].rearrange("c (h x) -> c h x", x=128)
                        # A-diagonal tiles (cols [0,W) of each head half)
                        nc.scalar.activation(
                            p3[:, :, 0:W], s3[:, :, 0:W], EXP, scale=SCALE
                        )
                        est["dve"] += 470.0
                        nc.vector.tensor_tensor(
                            p3[:, :, 0:W], p3[:, :, 0:W], mA, MULT
                        )
                        if gw > 1:
                            # B-diagonal right halves (cols [W+128, 2W))
                            nc.scalar.activation(
                                p3[:, :, W + 128:2 * W],
                                s3[:, :, W + 128:2 * W],
                                EXP, scale=SCALE,
                            )
                            est["dve"] += 330.0
                            nc.vector.tensor_tensor(
                                p3[:, :, W + 128:2 * W],
                                p3[:, :, W + 128:2 * W], mL, MULT,
                            )
                    flush_pending()
                    pending.append((qb, kts, p, pvAB, gi == 0,
                                    gi == len(groups) - 1))
            flush_pending(force=True)
        drain_outs()

        pv_pool.release()
        ps_pool.release()
        o_pool.release()
        p_pool.release()
        vx_pool.release()
        kq_pool.release()
        const_pool.release()

    nc.compile()
    return nc


def _get_nc():
    if "nc" not in _CACHE:
        _CACHE["nc"] = _build_nc()
    return _CACHE["nc"]


def _prep_inputs(q, k, v):
    qf = np.ascontiguousarray(np.asarray(q, dtype=np.float32)).reshape(BH, S, D)
    kf = np.ascontiguousarray(np.asarray(k, dtype=np.float32)).reshape(BH, S, D)
    vf = np.ascontiguousarray(np.asarray(v, dtype=np.float32)).reshape(BH, S, D)
    vx = np.empty((BH, S, D + 1), np.float16)
    vx[:, :, :D] = vf
    vx[:, :, D] = 1.0
    # pack v partition-major: [BH, NKT, 128, D+1] -> [BH, 128, NKT*(D+1)]
    vp = np.ascontiguousarray(
        vx.reshape(BH, NKT, 128, D + 1).transpose(0, 2, 1, 3)
    ).reshape(BH, 128, NKT * (D + 1))
    qt = qf.transpose(0, 2, 1).astype(np.float16)
    kt = kf.transpose(0, 2, 1).astype(np.float16)
    in_maps = []
    for c in range(NCORES):
        sl = slice(HPC * c, HPC * (c + 1))
        in_maps.append({
            "qt": np.ascontiguousarray(qt[sl]),
            "kt": np.ascontiguousarray(kt[sl]),
            "v": np.ascontiguousarray(vp[sl]),
        })
    return in_maps


def _postprocess(results):
    out = np.empty((B, H, S, D), np.float32)
    for c in range(NCORES):
        ot = results[c]["outT"]  # [HPC, D+1, S]
        o = (ot[:, :D, :] / ot[:, D:D + 1, :]).transpose(0, 2, 1)  # [HPC, S, D]
        for i in range(HPC):
            bh = HPC * c + i
            out[bh // H, bh % H] = o[i]
    return out


def run(q, k, v, trace=False):
    from concourse.bass_utils import run_bass_kernel_spmd

    nc = _get_nc()
    in_maps = _prep_inputs(q, k, v)
    res = run_bass_kernel_spmd(
        nc, in_maps, core_ids=list(range(NCORES)), trace=trace
    )
    return _postprocess(res.results), res


def kernel(q, k, v):
    out, _ = run(q, k, v, trace=False)
    return out
